# revision 60
# baseline (speedup 1.0000x reference)
"""ConvLSTM cell (complex-valued gates) on 8 TRN2 NeuronCores.

Strategy
--------
Data-parallel over batch: 16 images -> 2 per core. Per core, each gate's
complex 3x3 conv is computed as shifted matmuls accumulated in PSUM:

    out[128, 512] += lhsT[128in, 128out].T @ z_shift[128in, 512]

PE work is cut 54 -> 45 matmul-columns per tap-position via Gauss's
3-multiply complex trick on the (i, c) gate PAIR. All gates convolve the
same z, so the three half-width (64-out) Gauss terms of gates i and c
pack into full 128-out matmuls (27 taps/pair vs 36 direct):

    k1 = conv(zs, [Wr_i | Wr_c])          zs = zr + zi (host-prepped)
    k2 = conv(zr, [(Wi-Wr)_i | (Wi-Wr)_c])
    k3 = conv(zi, [(Wr+Wi)_i | (Wr+Wi)_c])
    Re = k1 - k3, Im = k1 + k2            (VectorE, one PSUM operand each)

The pair matmuls run k-major (all k1 taps, then k3, then k2) so the
k1 PSUM->SBUF copy and the Re combine execute UNDER the remaining pair
matmuls (in-order engines + data deps give the overlap for free), and
only Im's combine trails the last pair matmul. The partition-crossed
duplicates the complex epilogue needs (CTs = [cti; -ctr] from CT,
X = [-ti; tr] from T) are VectorE copy/negate (4x-mode fp16, ~330ns)
instead of extra ScalarE tanh passes (1.14us each): ScalarE (the
tail-critical engine at ~12us/tile measured) drops to ~7us/tile.
GpSimd was tried for these and is catastrophically slow (14.7us/op).

The o gate stays direct (18 taps: [Wr|Wi] / [-Wi|Wr]) since it has no
partner to share half-width matmuls with, and must remain separable for
the schedule tail. PSUM: 3 pair tiles + 1 o tile x 2 banks = all 8 banks.

All matmul operands are fp16 (full PE speed; validated ~2e-3 worst-case
scale-relative error end to end vs the fp32 reference). ScalarE applies
sigmoid/tanh with per-channel bias fused (crossing partition bases where
the complex epilogue needs it). VectorE does the complex elementwise
update in fp16 (2x mode). x (*) c_prev is precomputed on the host and
added on-chip. Outputs leave as fp16 and are upcast on host.

The spatial dim is processed in macro-tiles per core of up to [128, 1024]
(1 batch x 16 rows x 64 cols). z is kept resident in SBUF, zero-padded to
66x66 on the host (zr/zi/zs stacked in one tensor so each chunk is a
single DMA) so conv taps are plain shifted access patterns.

Schedule shape (measured on hw): the PE matmul burst is gap-free at ~218ns
per 512-col matmul; the wins over the naive schedule are at the edges:
 - startup: weights/z split into small first-need chunks so the first
   matmul starts ~2us earlier (trigger order == DGE service order);
 - tail: the last 16 rows run the i+c pair first (epilogue in two 512-col
   halves), then two 8-row o-gate-only slices whose PSUM lives in the
   freed pair banks, so the chain after the final matmul is just
   sigmoid -> 4 muls -> sub -> h DMA.
"""
import sys
import numpy as np

sys.path.insert(0, "/opt/trn_rl_repo")

P = 128          # partitions / channels (64 real + 64 imag)
HALF = 64
B = 16           # full batch
N_CORES = 8
B_CORE = B // N_CORES   # batch per core
H = W = 64
HP = WP = 66     # padded spatial
N_RB = 4         # row-blocks per batch (16 rows each)
MACRO = 16 * W   # 1024 columns per macro tile
NSLOT = 45       # packed weight slots: 27 Gauss-pair (i,c) + 18 direct o
O_BASE = 27      # first o-gate slot

_CACHE = {}


def _apply_drain_patch(tile_mod):
    """The kernel-tail drain aggregates one wait per live proc-semaphore, but
    walrus rejects instructions with more than a few sync waits. Split the
    tail waits across a chain of single-wait drains."""
    if getattr(tile_mod.TileContext, "_drain_patched", False):
        return

    def _patched(self, tick_clock, wait_clock):
        ScopedClock = tile_mod.ScopedClock
        nc = self.nc
        drain_inst = nc.sync.drain()
        wait_clock.add_sem_waits(
            drain_inst.ins, ScopedClock({None: tick_clock.global_clock})
        )
        NW = 3
        si = drain_inst.ins.sync_info
        if si is not None and si.on_wait and len(si.on_wait) > NW:
            conds = list(si.on_wait)
            si.on_wait = conds[:NW]
            rest = conds[NW:]
            while rest:
                extra = nc.sync.drain()
                esi = extra.ins.sync_info
                if esi is None:
                    import bass_rust
                    extra.ins.sync_info = bass_rust.SyncInfo(
                        on_wait=rest[:NW], on_update=[])
                else:
                    esi.on_wait = rest[:NW]
                rest = rest[NW:]

        nc.all_engine_barrier()
        assert self.sems is not None
        popped = nc._tile_sem_poison_stack.pop()
        assert popped is self._sem_poison
        nc.clear_and_free_semaphores(list(self.sems.allocated().values()))
        nc.all_engine_barrier()

    tile_mod.TileContext._drain_and_barrier = _patched
    tile_mod.TileContext._drain_patched = True


def _split_excess_waits(nc, max_waits=1):
    """walrus's per-instruction sync-wait slots are tight (1 for some ISA
    structs). Hoist excess waits into same-engine no-ops inserted directly
    before the instruction — identical semantics, per-engine order kept."""
    import concourse.mybir as mybir
    n_new = 0
    for fn in nc.m.functions:
        for bb in fn.blocks:
            il = bb.instructions
            out = []
            for inst in il:
                si = inst.sync_info
                if si is not None and si.on_wait and len(si.on_wait) > max_waits:
                    conds = list(si.on_wait)
                    si.on_wait = conds[:max_waits]
                    rest = conds[max_waits:]
                    for j in range(0, len(rest), max_waits):
                        nop = mybir.InstNoOp(
                            name=f"{inst.name}_w{j}",
                            sync_info=mybir.SyncInfo(
                                on_wait=rest[j:j + max_waits], on_update=[]),
                            bass_nofuse=True,
                            engine=inst.engine,
                        )
                        out.append(nop)
                        n_new += 1
                out.append(inst)
            if n_new:
                il[:] = out
    return n_new


def _build_program():
    import concourse.bass as bass
    import concourse.tile as tile
    from concourse import mybir
    from contextlib import ExitStack

    _apply_drain_patch(tile)
    fp16 = mybir.dt.float16
    f32 = mybir.dt.float32
    Sigmoid = mybir.ActivationFunctionType.Sigmoid
    Tanh = mybir.ActivationFunctionType.Tanh

    nc = bass.Bass("TRN2", target_bir_lowering=False, debug=False)
    zc_d = nc.dram_tensor("zc", [P, 3, B_CORE, HP, WP], fp16,
                          kind="ExternalInput").ap()
    w_d = nc.dram_tensor("wts", [P, NSLOT, P], fp16, kind="ExternalInput").ap()
    b_d = nc.dram_tensor("bias", [P, 4], f32, kind="ExternalInput").ap()
    xc_d = nc.dram_tensor("xc", [P, B_CORE, H, W], fp16, kind="ExternalInput").ap()
    h_d = nc.dram_tensor("h_out", [P, B_CORE, H, W], fp16, kind="ExternalOutput").ap()
    c_d = nc.dram_tensor("c_out", [P, B_CORE, H, W], fp16, kind="ExternalOutput").ap()

    # padded-row chunks (overlapping). For b=0 the 'A' range is split so the
    # first macro-tile's matmuls gate on a 170KB transfer instead of 300KB.
    Z_CHUNKS = {
        0: {'A1': (0, 10), 'A2': (8, 10), 'B': (16, 18), 'C': (32, 34)},
        1: {'A': (0, 18), 'B': (16, 18), 'C': (32, 34)},
    }

    with tile.TileContext(nc) as tc, ExitStack() as ctx:
        const = ctx.enter_context(tc.tile_pool(name="const", bufs=1))
        bias_s = const.tile([P, 4], f32)

        # NOTE: pre-warming the PE with dummy matmuls during the DMA wait was
        # tried and makes things WORSE: with warmup the PE clock settles ~20%
        # lower (259ns vs 215ns per 512-col matmul) for the entire burst.
        # The natural idle->burst pattern reaches the higher sustained clock.

        # chunked input loads, in first-consumption order, so the first
        # macro-tile's matmuls start after ~0.7MB of DMA instead of ~7MB
        w_g = {'p': [], 'o': []}   # group -> list of (tile, slot_off, nslots)
        z_ch = {}
        xc_t = {}     # (b, half) -> tile

        def load_w(grp, off, nt):
            base = O_BASE if grp == 'o' else 0
            wt = const.tile([P, nt, P], fp16, name=f"w_{grp}{off}")
            nc.sync.dma_start(wt[:], w_d[:, base + off:base + off + nt, :])
            w_g[grp].append((wt, off, nt))

        def defer(dm, after):
            if after is not None:
                # hold the transfer back until the anchor matmul retires so
                # it can't steal HBM bandwidth from earlier-needed loads
                tile.add_dep_helper(dm.ins, after,
                                    reason="defer non-critical load")

        def load_z(b, ch, after=None):
            # all 3 comps in ONE DMA: a staggered per-comp start was tried
            # and the resulting stop-start first tile locks the PE DVFS at
            # the LOW clock (262ns/matmul vs 218) for the entire kernel.
            row0, nr = Z_CHUNKS[b][ch]
            t = const.tile([P, 3, nr, WP], fp16, name=f"z_{b}_{ch}")
            defer(nc.sync.dma_start(t[:], zc_d[:, :, b, row0:row0 + nr, :]),
                  after)
            z_ch[(b, ch)] = t

        def load_z_comp(b, ch, c):
            # startup-only: one comp as its own tile so the first k1 matmuls
            # gate on just zs; the k-major order leaves ~4us (DVFS ramp)
            # before zi/zr are touched, so the split cannot stutter the PE.
            row0, nr = Z_CHUNKS[b][ch]
            t = const.tile([P, nr, WP], fp16, name=f"z_{b}_{ch}_{c}")
            nc.sync.dma_start(t[:], zc_d[:, c, b, row0:row0 + nr, :])
            z_ch.setdefault((b, ch), [None, None, None])[c] = t

        def load_xc(b, half, after=None):
            t = const.tile([P, 32, W], fp16, name=f"xc_{b}_{half}")
            defer(nc.sync.dma_start(
                t[:], xc_d[:, b, half * 32:half * 32 + 32, :]), after)
            xc_t[(b, half)] = t

        # upfront: only what macro-tile (b=0, rb=0/1) needs; the rest is
        # emitted mid-loop so the DGE serves the critical transfers first.
        # Trigger order == DGE service order. The first matmul gates on
        # w_p[0:9] + z_A1 (~0.8MB); later chunks arrive under the burst —
        # the start must be DENSE (see the DVFS note above).
        load_w('p', 0, 3)      # k1 taps 0-2: first matmul gates on 96KB of w
        load_z_comp(0, 'A1', 2)   # zs: all the z the k1 group needs
        load_w('p', 3, 6)      # k1 taps 3-8, lands under the first matmuls
        load_z_comp(0, 'A1', 1)   # zi (k3, first touched at matmul 9)
        load_w('p', 9, 9)      # k3
        load_z_comp(0, 'A1', 0)   # zr (k2, matmul 18)
        load_w('p', 18, 9)     # k2
        nc.sync.dma_start(bias_s[:], b_d[:])
        load_w('o', 0, 9)
        load_z(0, 'A2')
        load_w('o', 9, 9)
        load_xc(0, 0)

        ps_k1 = ctx.enter_context(tc.tile_pool(name="ps_k1", bufs=1, space="PSUM"))
        ps_k2 = ctx.enter_context(tc.tile_pool(name="ps_k2", bufs=1, space="PSUM"))
        ps_k3 = ctx.enter_context(tc.tile_pool(name="ps_k3", bufs=1, space="PSUM"))
        # NOTE: ps_o bufs=2 was tried to overlap the tail o-tiles and is
        # ~1.3us WORSE on hardware (fully-packed PSUM hurts); keep bufs=1.
        ps_o = ctx.enter_context(tc.tile_pool(name="ps_o", bufs=1, space="PSUM"))
        work = ctx.enter_context(tc.tile_pool(name="work", bufs=2))

        def find_chunk(b, r0, nrows):
            ch = next(c for c, (row0, nr) in Z_CHUNKS[b].items()
                      if r0 >= row0 and r0 + nrows + 2 <= row0 + nr)
            return ch, Z_CHUNKS[b][ch][0]

        def lookup_w(grp, m):
            wt, woff = next((t, o) for t, o, nt in w_g[grp]
                            if o <= m < o + nt)
            return wt, woff

        def z_rhs(b, ch, comp, rows, cs):
            z = z_ch[(b, ch)]
            if isinstance(z, list):
                return z[comp][:, rows, cs]
            return z[:, comp, rows, cs]

        def k_group(pool, tag, slot0, zcomp, b, r0, nrows, roff, ch, last_mm):
            """One Gauss k-term: 9-tap accumulation group in one PSUM tile."""
            cols = nrows * W
            pt = pool.tile([P, cols], f32, tag=tag, name=tag)
            for t in range(9):
                kh, kw = t // 3, t % 3
                m = slot0 + t
                wt, woff = lookup_w('p', m)
                for c0 in range(0, cols, 512):
                    nr = min(cols - c0, 512) // W
                    r0h = r0 + c0 // W - roff
                    mm = nc.tensor.matmul(
                        pt[:, c0:c0 + nr * W],
                        wt[:, m - woff, :],
                        z_rhs(b, ch, zcomp,
                              slice(r0h + kh, r0h + kh + nr),
                              slice(kw, kw + 64)),
                        start=(t == 0), stop=(t == 8),
                    )
                    last_mm[0] = mm.ins
            return pt

        def conv_o(b, r0, nrows, last_mm, pool=None, tag="pt_o"):
            cols = nrows * W
            ch, roff = find_chunk(b, r0, nrows)
            pt = (pool or ps_o).tile([P, cols], f32, tag=tag, name=tag)
            for k in range(18):
                kh, kw, ci = k // 6, (k // 2) % 3, k % 2
                m = (kh * 3 + kw) * 2 + ci
                wt, woff = lookup_w('o', m)
                for c0 in range(0, cols, 512):
                    nr = min(cols - c0, 512) // W
                    r0h = r0 + c0 // W - roff
                    mm = nc.tensor.matmul(
                        pt[:, c0:c0 + nr * W],
                        wt[:, m - woff, :],
                        z_rhs(b, ch, ci,
                              slice(r0h + kh, r0h + kh + nr),
                              slice(kw, kw + 64)),
                        start=(k == 0), stop=(k == 17),
                    )
                    last_mm[0] = mm.ins
            return pt

        def o_tile(b, r0, nrows, par_r0, halves, pool=None, tag="pt_o",
                   split_out=False):
            """o-gate conv + h epilogue for a row slice of an earlier
            do_o=False macro tile. Kept tiny at the schedule tail so the
            post-last-matmul chain is just sigmoid -> 4 muls -> sub -> DMA.
            Each tail slice gets its own freed pair-PSUM pool so the slice
            convs don't serialize behind the previous slice's sigmoid."""
            cols = nrows * W
            s0, step, T, X = next(
                h for h in halves
                if h[0] <= r0 - par_r0 and r0 - par_r0 + nrows <= h[0] + h[1])
            off = (r0 - par_r0 - s0) * W
            last_mm = [None]
            pt_o = conv_o(b, r0, nrows, last_mm, pool=pool, tag=tag)
            O = work.tile([P, cols], fp16, tag="O")        # [or; oi]
            o_act = nc.scalar.activation(O[:], pt_o[:], Sigmoid,
                                         bias=bias_s[:, 2:3])
            Q1 = work.tile([P, cols], fp16, tag="Q1")      # [or*tr ; oi*tr]
            nc.vector.tensor_mul(Q1[0:HALF, :], O[0:HALF, :],
                                 T[0:HALF, off:off + cols])
            nc.vector.tensor_mul(Q1[HALF:P, :], O[HALF:P, :],
                                 X[HALF:P, off:off + cols])
            Q2 = work.tile([P, cols], fp16, tag="Q2")      # [oi*ti ; -or*ti]
            nc.vector.tensor_mul(Q2[0:HALF, :], O[HALF:P, :],
                                 T[HALF:P, off:off + cols])
            nc.vector.tensor_mul(Q2[HALF:P, :], O[0:HALF, :],
                                 X[0:HALF, off:off + cols])
            hnew = work.tile([P, cols], fp16, tag="hnew")
            if split_out:
                # very last slice: sub + DMA in two halves so the first
                # h transfer starts earlier and the final one is smaller
                # (exec end tracks the last DMA packet + fixed barrier)
                hc, hr = cols // 2, nrows // 2
                nc.vector.tensor_sub(hnew[:, 0:hc], Q1[:, 0:hc], Q2[:, 0:hc])
                nc.sync.dma_start(h_d[:, b, r0:r0 + hr, :], hnew[:, 0:hc])
                nc.vector.tensor_sub(hnew[:, hc:], Q1[:, hc:], Q2[:, hc:])
                nc.sync.dma_start(h_d[:, b, r0 + hr:r0 + nrows, :],
                                  hnew[:, hc:])
            else:
                nc.vector.tensor_sub(hnew[:], Q1[:], Q2[:])
                nc.sync.dma_start(h_d[:, b, r0:r0 + nrows, :], hnew[:])
            return o_act.ins

        def pair_epilogue(b, r0, pks, s0, snr, sfx="", x_on_scalar=False):
            """cnew/T/X for rows [r0+s0, r0+s0+snr) of a finished pair conv.
            The tail tile runs this in two 512-col halves so the first
            half's T/X feed the first o-slice while the second computes."""
            pk1, pk2, pk3 = pks
            cols = snr * W
            sl = slice(s0 * W, s0 * W + cols)
            k1s = work.tile([P, cols], fp16, tag="k1s" + sfx)
            nc.scalar.copy(k1s[:], pk1[:, sl])
            sm = work.tile([P, cols], fp16, tag="sm" + sfx)   # [Re_i; Re_c]
            nc.vector.tensor_sub(sm[:], k1s[:], pk3[:, sl])
            sp = work.tile([P, cols], fp16, tag="sp" + sfx)   # [Im_i; Im_c]
            nc.vector.tensor_add(sp[:], k1s[:], pk2[:, sl])

            # activations: ScalarE crosses partition bases; the pure
            # duplicate/negate halves (CTs from CT, X from T below) are
            # VectorE copy/negate, keeping ScalarE off the tail path.
            CT = work.tile([P, cols], fp16, tag="CT" + sfx)   # [ctr; cti]
            nc.scalar.activation(CT[0:HALF, :], sm[HALF:P, :], Tanh,
                                 bias=bias_s[HALF:P, 0:1])
            nc.scalar.activation(CT[HALF:P, :], sp[HALF:P, :], Tanh,
                                 bias=bias_s[HALF:P, 1:2])
            CTs = work.tile([P, cols], fp16, tag="CTs" + sfx)  # [cti; -ctr]
            nc.vector.tensor_copy(CTs[0:HALF, :], CT[HALF:P, :])
            nc.vector.tensor_scalar_mul(CTs[HALF:P, :], CT[0:HALF, :], -1.0)
            I = work.tile([P, cols], fp16, tag="I" + sfx)      # [ir; ii]
            nc.scalar.activation(I[0:HALF, :], sm[0:HALF, :], Sigmoid,
                                 bias=bias_s[0:HALF, 0:1])
            nc.scalar.activation(I[HALF:P, :], sp[0:HALF, :], Sigmoid,
                                 bias=bias_s[0:HALF, 1:2])

            # i (*) ct (complex): product halves written to base-0/base-64 so
            # every TensorTensor keeps same-base inputs
            P1 = work.tile([P, cols], fp16, tag="P1" + sfx)   # [ir*ctr; ir*cti]
            nc.vector.tensor_mul(P1[0:HALF, :], I[0:HALF, :], CT[0:HALF, :])
            nc.vector.tensor_mul(P1[HALF:P, :], I[0:HALF, :], CTs[0:HALF, :])
            P2 = work.tile([P, cols], fp16, tag="P2" + sfx)   # [ii*cti; -ii*ctr]
            nc.vector.tensor_mul(P2[0:HALF, :], I[HALF:P, :], CT[HALF:P, :])
            nc.vector.tensor_mul(P2[HALF:P, :], I[HALF:P, :], CTs[HALF:P, :])
            tmp = work.tile([P, cols], fp16, tag="tmp" + sfx)
            nc.vector.tensor_sub(tmp[:], P1[:], P2[:])

            row0 = r0 + s0
            xch = xc_t[(b, row0 // 32)]
            xr0 = row0 % 32
            cnew = work.tile([P, cols], fp16, tag="cnew" + sfx)
            nc.vector.tensor_add(cnew[:], xch[:, xr0:xr0 + snr, :], tmp[:])
            nc.sync.dma_start(c_d[:, b, row0:row0 + snr, :], cnew[:])

            # the real/imag crossing for the h update is done on T (ready
            # before the o-gate matmuls finish) instead of on O, so only ONE
            # activation remains on the post-last-matmul critical path.
            T = work.tile([P, cols], fp16, tag="T" + sfx)     # [tr; ti]
            nc.scalar.activation(T[:], cnew[:], Tanh)
            X = work.tile([P, cols], fp16, tag="X" + sfx)     # [-ti; tr]
            if x_on_scalar:
                # tail tiles: VectorE is the serial bottleneck after the
                # last matmul while ScalarE idles -> derive X by crossed
                # tanh from cnew (tanh(-x) = -tanh(x)) on ScalarE instead
                nc.scalar.activation(X[0:HALF, :], cnew[HALF:P, :], Tanh,
                                     scale=-1.0)
                nc.scalar.activation(X[HALF:P, :], cnew[0:HALF, :], Tanh)
            else:
                nc.vector.tensor_scalar_mul(X[0:HALF, :], T[HALF:P, :], -1.0)
                nc.vector.tensor_copy(X[HALF:P, :], T[0:HALF, :])
            return T, X

        def macro_tile(b, r0, nrows, do_o=True, splits=1):
            cols = nrows * W
            last_mm = [None]

            # Gauss pair, k-major: the k1s copy runs under the k3 matmuls
            # and sm under the k2 matmuls (in-order engines + data deps);
            # only sp trails the final pair matmul. TensorTensor may read
            # only ONE operand from PSUM, hence the k1 PSUM->SBUF copy.
            ch, roff = find_chunk(b, r0, nrows)
            pk1 = k_group(ps_k1, "pt_k1", 0, 2, b, r0, nrows, roff, ch, last_mm)
            pk3 = k_group(ps_k3, "pt_k3", 9, 1, b, r0, nrows, roff, ch, last_mm)
            pk2 = k_group(ps_k2, "pt_k2", 18, 0, b, r0, nrows, roff, ch, last_mm)
            pks = (pk1, pk2, pk3)
            step = nrows // splits
            halves = []
            for si in range(splits):
                T, X = pair_epilogue(b, r0, pks, si * step, step,
                                     sfx="" if splits == 1 else str(si))
                halves.append((si * step, step, T, X))

            if do_o:
                T, X = halves[0][2], halves[0][3]
                pt_o = conv_o(b, r0, nrows, last_mm)
                O = work.tile([P, cols], fp16, tag="O")    # [or; oi]
                nc.scalar.activation(O[:], pt_o[:], Sigmoid,
                                     bias=bias_s[:, 2:3])
                Q1 = work.tile([P, cols], fp16, tag="Q1")  # [or*tr ; oi*tr]
                nc.vector.tensor_mul(Q1[0:HALF, :], O[0:HALF, :], T[0:HALF, :])
                nc.vector.tensor_mul(Q1[HALF:P, :], O[HALF:P, :], X[HALF:P, :])
                Q2 = work.tile([P, cols], fp16, tag="Q2")  # [oi*ti ; -or*ti]
                nc.vector.tensor_mul(Q2[0:HALF, :], O[HALF:P, :], T[HALF:P, :])
                nc.vector.tensor_mul(Q2[HALF:P, :], O[0:HALF, :], X[0:HALF, :])
                hnew = work.tile([P, cols], fp16, tag="hnew")
                nc.vector.tensor_sub(hnew[:], Q1[:], Q2[:])
                nc.sync.dma_start(h_d[:, b, r0:r0 + nrows, :], hnew[:])
            return last_mm[0], halves

        # first tile small (PE starts on less DMA'd data), last tiles small
        # (short post-matmul epilogue chain); z-chunk loads two tiles ahead
        SCHEDULE = [(0, 0, 8), (0, 8, 8), (0, 16, 16), (0, 32, 16),
                    (0, 48, 16), (1, 0, 16), (1, 16, 16), (1, 32, 16)]
        deferred = {1: [(0, 'B')], 2: [(0, 'C'), ('xc', 0, 1)],
                    3: [(1, 'A')], 4: [(1, 'B'), ('xc', 1, 0)],
                    5: [(1, 'C'), ('xc', 1, 1)]}
        anchor = None
        for tidx, (b, r0, nrows) in enumerate(SCHEDULE):
            for item in deferred.pop(tidx, []):
                if item[0] == 'xc':
                    load_xc(item[1], item[2], after=anchor)
                else:
                    load_z(item[0], item[1], after=anchor)
            anchor, _ = macro_tile(b, r0, nrows)

        # tail: the last 8 rows run the i+c pair only, epilogue in two
        # 256-col halves interleaved with tiny o-gate slices (each slice's
        # PSUM in a freed pair pool, so the convs never serialize behind a
        # sigmoid). Minimal VectorE work is gated on the final matmuls; the
        # chain after the very last one is sigmoid -> 4 muls -> sub -> DMA.
        last_mm = [None]
        ch, roff = find_chunk(1, 48, 16)
        pk1 = k_group(ps_k1, "pt_k1", 0, 2, 1, 48, 16, roff, ch, last_mm)
        pk3 = k_group(ps_k3, "pt_k3", 9, 1, 1, 48, 16, roff, ch, last_mm)
        pk2 = k_group(ps_k2, "pt_k2", 18, 0, 1, 48, 16, roff, ch, last_mm)
        pks = (pk1, pk2, pk3)
        h0 = [(0, 8) + pair_epilogue(1, 48, pks, 0, 8, sfx="0",
                                     x_on_scalar=True)]
        o_tile(1, 48, 8, 48, h0, pool=ps_k1, tag="pt_k1")
        h1 = [(8, 8) + pair_epilogue(1, 48, pks, 8, 8, sfx="1",
                                     x_on_scalar=True)]
        o_tile(1, 56, 8, 48, h1, pool=ps_k2, tag="pt_k2", split_out=True)

    _split_excess_waits(nc)
    return nc


def _prep_inputs(inputs):
    """Host-side shard + layout prep. Returns per-core in_maps."""
    f16 = np.float16
    x = np.asarray(inputs['x'], np.float32)
    h_prev = np.asarray(inputs['h_prev'], np.float32)
    c_prev = np.asarray(inputs['c_prev'], np.float32)

    xr, xi = x[:, :HALF], x[:, HALF:]
    hr, hi = h_prev[:, :HALF], h_prev[:, HALF:]
    cr, ci = c_prev[:, :HALF], c_prev[:, HALF:]

    # combined conv input, channel-major, zero-padded: [128, 3, B, 66, 66]
    # comps: 0 = zr, 1 = zi, 2 = zs = zr + zi (for the Gauss k1 term)
    def prep_z(a, b):
        z = np.concatenate([a, b], axis=1).transpose(1, 0, 2, 3)
        return np.pad(z, ((0, 0), (0, 0), (1, 1), (1, 1)))
    zr = prep_z(xr, hr)
    zi = prep_z(xi, hi)
    zall = np.stack([zr, zi, zr + zi], axis=1).astype(f16)

    # x (*) c_prev (complex elementwise), channel-major fp16: [128, B, 64, 64]
    xc = np.concatenate([xr * cr - xi * ci, xr * ci + xi * cr],
                        axis=1).transpose(1, 0, 2, 3).astype(f16)

    # packed gate weights [cin 128, 45, cout 128] fp16, k-major:
    #  slots 0-8 k1 taps [Wr_i | Wr_c], 9-17 k3 [(Wr+Wi)_i | (Wr+Wi)_c],
    #  18-26 k2 [(Wi-Wr)_i | (Wi-Wr)_c] for the Gauss (i, c) pair;
    #  slots 27 + t*2 + comp for direct o: [Wr|Wi] / [-Wi|Wr]
    W4 = {gn: (np.asarray(inputs['Wr_' + gn], np.float32),
               np.asarray(inputs['Wi_' + gn], np.float32))
          for gn in ('i', 'c', 'o')}
    wts = np.empty((NSLOT, P, P), np.float32)
    for t in range(9):
        kh, kw = t // 3, t % 3
        for gi, gn in enumerate(('i', 'c')):
            Wr, Wi = W4[gn]
            sl = slice(gi * HALF, gi * HALF + HALF)
            wts[t, :, sl] = Wr[:, :, kh, kw].T
            wts[9 + t, :, sl] = (Wr + Wi)[:, :, kh, kw].T
            wts[18 + t, :, sl] = (Wi - Wr)[:, :, kh, kw].T
        Wr, Wi = W4['o']
        wts[O_BASE + t * 2 + 0, :, :HALF] = Wr[:, :, kh, kw].T
        wts[O_BASE + t * 2 + 0, :, HALF:] = Wi[:, :, kh, kw].T
        wts[O_BASE + t * 2 + 1, :, :HALF] = -Wi[:, :, kh, kw].T
        wts[O_BASE + t * 2 + 1, :, HALF:] = Wr[:, :, kh, kw].T
    wts = np.ascontiguousarray(wts.transpose(1, 0, 2)).astype(f16)

    # bias cols: 0 -> sm sources [br_i; br_c], 1 -> sp sources [bi_i; bi_c],
    # 2 -> o gate [br_o; bi_o], 3 -> negated c-Re for CTs [0; -br_c]
    b4 = {k: np.asarray(inputs[k], np.float32)
          for k in ('br_i', 'bi_i', 'br_c', 'bi_c', 'br_o', 'bi_o')}
    bias = np.zeros((P, 4), np.float32)
    bias[:, 0] = np.concatenate([b4['br_i'], b4['br_c']])
    bias[:, 1] = np.concatenate([b4['bi_i'], b4['bi_c']])
    bias[:, 2] = np.concatenate([b4['br_o'], b4['bi_o']])
    bias[HALF:, 3] = -b4['br_c']

    in_maps = []
    for c in range(N_CORES):
        sl = slice(c * B_CORE, (c + 1) * B_CORE)
        in_maps.append({
            "zc": np.ascontiguousarray(zall[:, :, sl]),
            "wts": wts,
            "bias": bias,
            "xc": np.ascontiguousarray(xc[:, sl]),
        })
    return in_maps


def _gather_outputs(results):
    h_full = np.empty((B, P, H, W), np.float32)
    c_full = np.empty((B, P, H, W), np.float32)
    for c in range(N_CORES):
        sl = slice(c * B_CORE, (c + 1) * B_CORE)
        h_full[sl] = results[c]["h_out"].transpose(1, 0, 2, 3).astype(np.float32)
        c_full[sl] = results[c]["c_out"].transpose(1, 0, 2, 3).astype(np.float32)
    return h_full, c_full


def _run(inputs, trace=False, trace_kwargs=None):
    from concourse.bass_utils import run_bass_kernel_spmd

    if "nc" not in _CACHE:
        _CACHE["nc"] = _build_program()
    nc = _CACHE["nc"]
    in_maps = _prep_inputs(inputs)
    r = run_bass_kernel_spmd(nc, in_maps, list(range(N_CORES)),
                             trace=trace, trace_kwargs=trace_kwargs or {})
    return _gather_outputs(r.results), r


def kernel(**inputs):
    (h_full, c_full), _ = _run(inputs)
    return h_full, c_full



# revision 61
# speedup vs baseline: 1.0047x; 1.0047x over previous
"""ConvLSTM cell (complex-valued gates) on 8 TRN2 NeuronCores.

Strategy
--------
Data-parallel over batch: 16 images -> 2 per core. Per core, each gate's
complex 3x3 conv is computed as shifted matmuls accumulated in PSUM:

    out[128, 512] += lhsT[128in, 128out].T @ z_shift[128in, 512]

PE work is cut 54 -> 45 matmul-columns per tap-position via Gauss's
3-multiply complex trick on the (i, c) gate PAIR. All gates convolve the
same z, so the three half-width (64-out) Gauss terms of gates i and c
pack into full 128-out matmuls (27 taps/pair vs 36 direct):

    k1 = conv(zs, [Wr_i | Wr_c])          zs = zr + zi (host-prepped)
    k2 = conv(zr, [(Wi-Wr)_i | (Wi-Wr)_c])
    k3 = conv(zi, [(Wr+Wi)_i | (Wr+Wi)_c])
    Re = k1 - k3, Im = k1 + k2            (VectorE, one PSUM operand each)

The pair matmuls run k-major (all k1 taps, then k3, then k2) so the
k1 PSUM->SBUF copy and the Re combine execute UNDER the remaining pair
matmuls (in-order engines + data deps give the overlap for free), and
only Im's combine trails the last pair matmul. The partition-crossed
duplicates the complex epilogue needs (CTs = [cti; -ctr] from CT,
X = [-ti; tr] from T) are VectorE copy/negate (4x-mode fp16, ~330ns)
instead of extra ScalarE tanh passes (1.14us each): ScalarE (the
tail-critical engine at ~12us/tile measured) drops to ~7us/tile.
GpSimd was tried for these and is catastrophically slow (14.7us/op).

The o gate stays direct (18 taps: [Wr|Wi] / [-Wi|Wr]) since it has no
partner to share half-width matmuls with, and must remain separable for
the schedule tail. PSUM: 3 pair tiles + 1 o tile x 2 banks = all 8 banks.

All matmul operands are fp16 (full PE speed; validated ~2e-3 worst-case
scale-relative error end to end vs the fp32 reference). ScalarE applies
sigmoid/tanh with per-channel bias fused (crossing partition bases where
the complex epilogue needs it). VectorE does the complex elementwise
update in fp16 (2x mode). x (*) c_prev is precomputed on the host and
added on-chip. Outputs leave as fp16 and are upcast on host.

The spatial dim is processed in macro-tiles per core of up to [128, 1024]
(1 batch x 16 rows x 64 cols). z is kept resident in SBUF, zero-padded to
66x66 on the host (zr/zi/zs stacked in one tensor so each chunk is a
single DMA) so conv taps are plain shifted access patterns.

Schedule shape (measured on hw): the PE matmul burst is gap-free at ~218ns
per 512-col matmul; the wins over the naive schedule are at the edges:
 - startup: weights/z split into small first-need chunks so the first
   matmul starts ~2us earlier (trigger order == DGE service order);
 - tail: the last 16 rows run the i+c pair first (epilogue in two 512-col
   halves), then two 8-row o-gate-only slices whose PSUM lives in the
   freed pair banks, so the chain after the final matmul is just
   sigmoid -> 4 muls -> sub -> h DMA.
"""
import sys
import numpy as np

sys.path.insert(0, "/opt/trn_rl_repo")

P = 128          # partitions / channels (64 real + 64 imag)
HALF = 64
B = 16           # full batch
N_CORES = 8
B_CORE = B // N_CORES   # batch per core
H = W = 64
HP = WP = 66     # padded spatial
N_RB = 4         # row-blocks per batch (16 rows each)
MACRO = 16 * W   # 1024 columns per macro tile
NSLOT = 45       # packed weight slots: 27 Gauss-pair (i,c) + 18 direct o
O_BASE = 27      # first o-gate slot

_CACHE = {}


def _apply_drain_patch(tile_mod):
    """The kernel-tail drain aggregates one wait per live proc-semaphore, but
    walrus rejects instructions with more than a few sync waits. Split the
    tail waits across a chain of single-wait drains."""
    if getattr(tile_mod.TileContext, "_drain_patched", False):
        return

    def _patched(self, tick_clock, wait_clock):
        ScopedClock = tile_mod.ScopedClock
        nc = self.nc
        drain_inst = nc.sync.drain()
        wait_clock.add_sem_waits(
            drain_inst.ins, ScopedClock({None: tick_clock.global_clock})
        )
        NW = 3
        si = drain_inst.ins.sync_info
        if si is not None and si.on_wait and len(si.on_wait) > NW:
            conds = list(si.on_wait)
            si.on_wait = conds[:NW]
            rest = conds[NW:]
            while rest:
                extra = nc.sync.drain()
                esi = extra.ins.sync_info
                if esi is None:
                    import bass_rust
                    extra.ins.sync_info = bass_rust.SyncInfo(
                        on_wait=rest[:NW], on_update=[])
                else:
                    esi.on_wait = rest[:NW]
                rest = rest[NW:]

        nc.all_engine_barrier()
        assert self.sems is not None
        popped = nc._tile_sem_poison_stack.pop()
        assert popped is self._sem_poison
        nc.clear_and_free_semaphores(list(self.sems.allocated().values()))
        nc.all_engine_barrier()

    tile_mod.TileContext._drain_and_barrier = _patched
    tile_mod.TileContext._drain_patched = True


def _split_excess_waits(nc, max_waits=1):
    """walrus's per-instruction sync-wait slots are tight (1 for some ISA
    structs). Hoist excess waits into same-engine no-ops inserted directly
    before the instruction — identical semantics, per-engine order kept."""
    import concourse.mybir as mybir
    n_new = 0
    for fn in nc.m.functions:
        for bb in fn.blocks:
            il = bb.instructions
            out = []
            for inst in il:
                si = inst.sync_info
                if si is not None and si.on_wait and len(si.on_wait) > max_waits:
                    conds = list(si.on_wait)
                    si.on_wait = conds[:max_waits]
                    rest = conds[max_waits:]
                    for j in range(0, len(rest), max_waits):
                        nop = mybir.InstNoOp(
                            name=f"{inst.name}_w{j}",
                            sync_info=mybir.SyncInfo(
                                on_wait=rest[j:j + max_waits], on_update=[]),
                            bass_nofuse=True,
                            engine=inst.engine,
                        )
                        out.append(nop)
                        n_new += 1
                out.append(inst)
            if n_new:
                il[:] = out
    return n_new


def _build_program():
    import concourse.bass as bass
    import concourse.tile as tile
    from concourse import mybir
    from contextlib import ExitStack

    _apply_drain_patch(tile)
    fp16 = mybir.dt.float16
    f32 = mybir.dt.float32
    Sigmoid = mybir.ActivationFunctionType.Sigmoid
    Tanh = mybir.ActivationFunctionType.Tanh

    nc = bass.Bass("TRN2", target_bir_lowering=False, debug=False)
    zc_d = nc.dram_tensor("zc", [P, 3, B_CORE, HP, WP], fp16,
                          kind="ExternalInput").ap()
    w_d = nc.dram_tensor("wts", [P, NSLOT, P], fp16, kind="ExternalInput").ap()
    b_d = nc.dram_tensor("bias", [P, 4], f32, kind="ExternalInput").ap()
    xc_d = nc.dram_tensor("xc", [P, B_CORE, H, W], fp16, kind="ExternalInput").ap()
    h_d = nc.dram_tensor("h_out", [P, B_CORE, H, W], fp16, kind="ExternalOutput").ap()
    c_d = nc.dram_tensor("c_out", [P, B_CORE, H, W], fp16, kind="ExternalOutput").ap()

    # padded-row chunks (overlapping). For b=0 the 'A' range is split so the
    # first macro-tile's matmuls gate on a 170KB transfer instead of 300KB.
    Z_CHUNKS = {
        0: {'A1': (0, 10), 'A2': (8, 10), 'B': (16, 18), 'C': (32, 34)},
        1: {'A': (0, 18), 'B': (16, 18), 'C': (32, 34)},
    }

    with tile.TileContext(nc) as tc, ExitStack() as ctx:
        const = ctx.enter_context(tc.tile_pool(name="const", bufs=1))
        bias_s = const.tile([P, 4], f32)

        # NOTE: pre-warming the PE with dummy matmuls during the DMA wait was
        # tried and makes things WORSE: with warmup the PE clock settles ~20%
        # lower (259ns vs 215ns per 512-col matmul) for the entire burst.
        # The natural idle->burst pattern reaches the higher sustained clock.

        # chunked input loads, in first-consumption order, so the first
        # macro-tile's matmuls start after ~0.7MB of DMA instead of ~7MB
        w_g = {'p': [], 'o': []}   # group -> list of (tile, slot_off, nslots)
        z_ch = {}
        xc_t = {}     # (b, half) -> tile

        def load_w(grp, off, nt):
            base = O_BASE if grp == 'o' else 0
            wt = const.tile([P, nt, P], fp16, name=f"w_{grp}{off}")
            nc.sync.dma_start(wt[:], w_d[:, base + off:base + off + nt, :])
            w_g[grp].append((wt, off, nt))

        def defer(dm, after):
            if after is not None:
                # hold the transfer back until the anchor matmul retires so
                # it can't steal HBM bandwidth from earlier-needed loads
                tile.add_dep_helper(dm.ins, after,
                                    reason="defer non-critical load")

        def load_z(b, ch, after=None):
            # all 3 comps in ONE DMA: a staggered per-comp start was tried
            # and the resulting stop-start first tile locks the PE DVFS at
            # the LOW clock (262ns/matmul vs 218) for the entire kernel.
            row0, nr = Z_CHUNKS[b][ch]
            t = const.tile([P, 3, nr, WP], fp16, name=f"z_{b}_{ch}")
            defer(nc.sync.dma_start(t[:], zc_d[:, :, b, row0:row0 + nr, :]),
                  after)
            z_ch[(b, ch)] = t

        def load_z_comp(b, ch, c):
            # startup-only: one comp as its own tile so the first k1 matmuls
            # gate on just zs; the k-major order leaves ~4us (DVFS ramp)
            # before zi/zr are touched, so the split cannot stutter the PE.
            row0, nr = Z_CHUNKS[b][ch]
            t = const.tile([P, nr, WP], fp16, name=f"z_{b}_{ch}_{c}")
            nc.sync.dma_start(t[:], zc_d[:, c, b, row0:row0 + nr, :])
            z_ch.setdefault((b, ch), [None, None, None])[c] = t

        def load_xc(b, half, after=None):
            t = const.tile([P, 32, W], fp16, name=f"xc_{b}_{half}")
            defer(nc.sync.dma_start(
                t[:], xc_d[:, b, half * 32:half * 32 + 32, :]), after)
            xc_t[(b, half)] = t

        # upfront: only what macro-tile (b=0, rb=0/1) needs; the rest is
        # emitted mid-loop so the DGE serves the critical transfers first.
        # Trigger order == DGE service order. The first matmul gates on
        # w_p[0:9] + z_A1 (~0.8MB); later chunks arrive under the burst —
        # the start must be DENSE (see the DVFS note above).
        load_w('p', 0, 3)      # k1 taps 0-2: first matmul gates on 96KB of w
        load_z_comp(0, 'A1', 2)   # zs: all the z the k1 group needs
        load_w('p', 3, 6)      # k1 taps 3-8, lands under the first matmuls
        load_z_comp(0, 'A1', 1)   # zi (k3, first touched at matmul 9)
        load_w('p', 9, 9)      # k3
        load_z_comp(0, 'A1', 0)   # zr (k2, matmul 18)
        load_w('p', 18, 9)     # k2
        nc.sync.dma_start(bias_s[:], b_d[:])
        load_w('o', 0, 9)
        load_z(0, 'A2')
        load_w('o', 9, 9)
        load_xc(0, 0)

        ps_k1 = ctx.enter_context(tc.tile_pool(name="ps_k1", bufs=1, space="PSUM"))
        ps_k2 = ctx.enter_context(tc.tile_pool(name="ps_k2", bufs=1, space="PSUM"))
        ps_k3 = ctx.enter_context(tc.tile_pool(name="ps_k3", bufs=1, space="PSUM"))
        # NOTE: ps_o bufs=2 was tried to overlap the tail o-tiles and is
        # ~1.3us WORSE on hardware (fully-packed PSUM hurts); keep bufs=1.
        ps_o = ctx.enter_context(tc.tile_pool(name="ps_o", bufs=1, space="PSUM"))
        work = ctx.enter_context(tc.tile_pool(name="work", bufs=2))

        def find_chunk(b, r0, nrows):
            ch = next(c for c, (row0, nr) in Z_CHUNKS[b].items()
                      if r0 >= row0 and r0 + nrows + 2 <= row0 + nr)
            return ch, Z_CHUNKS[b][ch][0]

        def lookup_w(grp, m):
            wt, woff = next((t, o) for t, o, nt in w_g[grp]
                            if o <= m < o + nt)
            return wt, woff

        def z_rhs(b, ch, comp, rows, cs):
            z = z_ch[(b, ch)]
            if isinstance(z, list):
                return z[comp][:, rows, cs]
            return z[:, comp, rows, cs]

        def k_group(pool, tag, slot0, zcomp, b, r0, nrows, roff, ch, last_mm):
            """One Gauss k-term: 9-tap accumulation group in one PSUM tile."""
            cols = nrows * W
            pt = pool.tile([P, cols], f32, tag=tag, name=tag)
            for t in range(9):
                kh, kw = t // 3, t % 3
                m = slot0 + t
                wt, woff = lookup_w('p', m)
                for c0 in range(0, cols, 512):
                    nr = min(cols - c0, 512) // W
                    r0h = r0 + c0 // W - roff
                    mm = nc.tensor.matmul(
                        pt[:, c0:c0 + nr * W],
                        wt[:, m - woff, :],
                        z_rhs(b, ch, zcomp,
                              slice(r0h + kh, r0h + kh + nr),
                              slice(kw, kw + 64)),
                        start=(t == 0), stop=(t == 8),
                    )
                    last_mm[0] = mm.ins
            return pt

        def conv_o(b, r0, nrows, last_mm, pool=None, tag="pt_o"):
            cols = nrows * W
            ch, roff = find_chunk(b, r0, nrows)
            pt = (pool or ps_o).tile([P, cols], f32, tag=tag, name=tag)
            for k in range(18):
                kh, kw, ci = k // 6, (k // 2) % 3, k % 2
                m = (kh * 3 + kw) * 2 + ci
                wt, woff = lookup_w('o', m)
                for c0 in range(0, cols, 512):
                    nr = min(cols - c0, 512) // W
                    r0h = r0 + c0 // W - roff
                    mm = nc.tensor.matmul(
                        pt[:, c0:c0 + nr * W],
                        wt[:, m - woff, :],
                        z_rhs(b, ch, ci,
                              slice(r0h + kh, r0h + kh + nr),
                              slice(kw, kw + 64)),
                        start=(k == 0), stop=(k == 17),
                    )
                    last_mm[0] = mm.ins
            return pt

        def o_tile(b, r0, nrows, par_r0, halves, pool=None, tag="pt_o",
                   split_out=False):
            """o-gate conv + h epilogue for a row slice of an earlier
            do_o=False macro tile. Kept tiny at the schedule tail so the
            post-last-matmul chain is just sigmoid -> 4 muls -> sub -> DMA.
            Each tail slice gets its own freed pair-PSUM pool so the slice
            convs don't serialize behind the previous slice's sigmoid."""
            cols = nrows * W
            s0, step, T, X = next(
                h for h in halves
                if h[0] <= r0 - par_r0 and r0 - par_r0 + nrows <= h[0] + h[1])
            off = (r0 - par_r0 - s0) * W
            last_mm = [None]
            pt_o = conv_o(b, r0, nrows, last_mm, pool=pool, tag=tag)
            O = work.tile([P, cols], fp16, tag="O")        # [or; oi]
            o_act = nc.scalar.activation(O[:], pt_o[:], Sigmoid,
                                         bias=bias_s[:, 2:3])
            Q1 = work.tile([P, cols], fp16, tag="Q1")      # [or*tr ; oi*tr]
            nc.vector.tensor_mul(Q1[0:HALF, :], O[0:HALF, :],
                                 T[0:HALF, off:off + cols])
            nc.vector.tensor_mul(Q1[HALF:P, :], O[HALF:P, :],
                                 X[HALF:P, off:off + cols])
            Q2 = work.tile([P, cols], fp16, tag="Q2")      # [oi*ti ; -or*ti]
            nc.vector.tensor_mul(Q2[0:HALF, :], O[HALF:P, :],
                                 T[HALF:P, off:off + cols])
            nc.vector.tensor_mul(Q2[HALF:P, :], O[0:HALF, :],
                                 X[0:HALF, off:off + cols])
            hnew = work.tile([P, cols], fp16, tag="hnew")
            if split_out:
                # very last slice: sub + DMA in two halves so the first
                # h transfer starts earlier and the final one is smaller
                # (exec end tracks the last DMA packet + fixed barrier)
                hc, hr = cols // 2, nrows // 2
                nc.vector.tensor_sub(hnew[:, 0:hc], Q1[:, 0:hc], Q2[:, 0:hc])
                nc.sync.dma_start(h_d[:, b, r0:r0 + hr, :], hnew[:, 0:hc])
                nc.vector.tensor_sub(hnew[:, hc:], Q1[:, hc:], Q2[:, hc:])
                nc.sync.dma_start(h_d[:, b, r0 + hr:r0 + nrows, :],
                                  hnew[:, hc:])
            else:
                nc.vector.tensor_sub(hnew[:], Q1[:], Q2[:])
                nc.sync.dma_start(h_d[:, b, r0:r0 + nrows, :], hnew[:])
            return o_act.ins

        def pair_epilogue(b, r0, pks, s0, snr, sfx="", x_on_scalar=False):
            """cnew/T/X for rows [r0+s0, r0+s0+snr) of a finished pair conv.
            The tail tile runs this in two 512-col halves so the first
            half's T/X feed the first o-slice while the second computes."""
            pk1, pk2, pk3 = pks
            cols = snr * W
            sl = slice(s0 * W, s0 * W + cols)
            k1s = work.tile([P, cols], fp16, tag="k1s" + sfx)
            nc.scalar.copy(k1s[:], pk1[:, sl])
            sm = work.tile([P, cols], fp16, tag="sm" + sfx)   # [Re_i; Re_c]
            nc.vector.tensor_sub(sm[:], k1s[:], pk3[:, sl])
            sp = work.tile([P, cols], fp16, tag="sp" + sfx)   # [Im_i; Im_c]
            nc.vector.tensor_add(sp[:], k1s[:], pk2[:, sl])

            # activations: ScalarE crosses partition bases; the pure
            # duplicate/negate halves (CTs from CT, X from T below) are
            # VectorE copy/negate, keeping ScalarE off the tail path.
            CT = work.tile([P, cols], fp16, tag="CT" + sfx)   # [ctr; cti]
            nc.scalar.activation(CT[0:HALF, :], sm[HALF:P, :], Tanh,
                                 bias=bias_s[HALF:P, 0:1])
            nc.scalar.activation(CT[HALF:P, :], sp[HALF:P, :], Tanh,
                                 bias=bias_s[HALF:P, 1:2])
            CTs = work.tile([P, cols], fp16, tag="CTs" + sfx)  # [cti; -ctr]
            nc.vector.tensor_copy(CTs[0:HALF, :], CT[HALF:P, :])
            nc.vector.tensor_scalar_mul(CTs[HALF:P, :], CT[0:HALF, :], -1.0)
            I = work.tile([P, cols], fp16, tag="I" + sfx)      # [ir; ii]
            nc.scalar.activation(I[0:HALF, :], sm[0:HALF, :], Sigmoid,
                                 bias=bias_s[0:HALF, 0:1])
            nc.scalar.activation(I[HALF:P, :], sp[0:HALF, :], Sigmoid,
                                 bias=bias_s[0:HALF, 1:2])

            # i (*) ct (complex): product halves written to base-0/base-64 so
            # every TensorTensor keeps same-base inputs
            P1 = work.tile([P, cols], fp16, tag="P1" + sfx)   # [ir*ctr; ir*cti]
            nc.vector.tensor_mul(P1[0:HALF, :], I[0:HALF, :], CT[0:HALF, :])
            nc.vector.tensor_mul(P1[HALF:P, :], I[0:HALF, :], CTs[0:HALF, :])
            P2 = work.tile([P, cols], fp16, tag="P2" + sfx)   # [ii*cti; -ii*ctr]
            nc.vector.tensor_mul(P2[0:HALF, :], I[HALF:P, :], CT[HALF:P, :])
            nc.vector.tensor_mul(P2[HALF:P, :], I[HALF:P, :], CTs[HALF:P, :])
            tmp = work.tile([P, cols], fp16, tag="tmp" + sfx)
            nc.vector.tensor_sub(tmp[:], P1[:], P2[:])

            row0 = r0 + s0
            xch = xc_t[(b, row0 // 32)]
            xr0 = row0 % 32
            cnew = work.tile([P, cols], fp16, tag="cnew" + sfx)
            nc.vector.tensor_add(cnew[:], xch[:, xr0:xr0 + snr, :], tmp[:])
            nc.sync.dma_start(c_d[:, b, row0:row0 + snr, :], cnew[:])

            # the real/imag crossing for the h update is done on T (ready
            # before the o-gate matmuls finish) instead of on O, so only ONE
            # activation remains on the post-last-matmul critical path.
            T = work.tile([P, cols], fp16, tag="T" + sfx)     # [tr; ti]
            nc.scalar.activation(T[:], cnew[:], Tanh)
            X = work.tile([P, cols], fp16, tag="X" + sfx)     # [-ti; tr]
            if x_on_scalar:
                # tail tiles: VectorE is the serial bottleneck after the
                # last matmul while ScalarE idles -> derive X by crossed
                # tanh from cnew (tanh(-x) = -tanh(x)) on ScalarE instead
                nc.scalar.activation(X[0:HALF, :], cnew[HALF:P, :], Tanh,
                                     scale=-1.0)
                nc.scalar.activation(X[HALF:P, :], cnew[0:HALF, :], Tanh)
            else:
                nc.vector.tensor_scalar_mul(X[0:HALF, :], T[HALF:P, :], -1.0)
                nc.vector.tensor_copy(X[HALF:P, :], T[0:HALF, :])
            return T, X

        def macro_tile(b, r0, nrows, do_o=True, splits=1):
            cols = nrows * W
            last_mm = [None]

            # Gauss pair, k-major: the k1s copy runs under the k3 matmuls
            # and sm under the k2 matmuls (in-order engines + data deps);
            # only sp trails the final pair matmul. TensorTensor may read
            # only ONE operand from PSUM, hence the k1 PSUM->SBUF copy.
            ch, roff = find_chunk(b, r0, nrows)
            pk1 = k_group(ps_k1, "pt_k1", 0, 2, b, r0, nrows, roff, ch, last_mm)
            pk3 = k_group(ps_k3, "pt_k3", 9, 1, b, r0, nrows, roff, ch, last_mm)
            pk2 = k_group(ps_k2, "pt_k2", 18, 0, b, r0, nrows, roff, ch, last_mm)
            pks = (pk1, pk2, pk3)
            step = nrows // splits
            halves = []
            for si in range(splits):
                T, X = pair_epilogue(b, r0, pks, si * step, step,
                                     sfx="" if splits == 1 else str(si))
                halves.append((si * step, step, T, X))

            if do_o:
                T, X = halves[0][2], halves[0][3]
                pt_o = conv_o(b, r0, nrows, last_mm)
                O = work.tile([P, cols], fp16, tag="O")    # [or; oi]
                nc.scalar.activation(O[:], pt_o[:], Sigmoid,
                                     bias=bias_s[:, 2:3])
                Q1 = work.tile([P, cols], fp16, tag="Q1")  # [or*tr ; oi*tr]
                nc.vector.tensor_mul(Q1[0:HALF, :], O[0:HALF, :], T[0:HALF, :])
                nc.vector.tensor_mul(Q1[HALF:P, :], O[HALF:P, :], X[HALF:P, :])
                Q2 = work.tile([P, cols], fp16, tag="Q2")  # [oi*ti ; -or*ti]
                nc.vector.tensor_mul(Q2[0:HALF, :], O[HALF:P, :], T[HALF:P, :])
                nc.vector.tensor_mul(Q2[HALF:P, :], O[0:HALF, :], X[0:HALF, :])
                hnew = work.tile([P, cols], fp16, tag="hnew")
                nc.vector.tensor_sub(hnew[:], Q1[:], Q2[:])
                nc.sync.dma_start(h_d[:, b, r0:r0 + nrows, :], hnew[:])
            return last_mm[0], halves

        # first tile small (PE starts on less DMA'd data), last tiles small
        # (short post-matmul epilogue chain); z-chunk loads two tiles ahead
        SCHEDULE = [(0, 0, 8), (0, 8, 8), (0, 16, 16), (0, 32, 16),
                    (0, 48, 16), (1, 0, 16), (1, 16, 16), (1, 32, 16)]
        deferred = {1: [(0, 'B')], 2: [(0, 'C'), ('xc', 0, 1)],
                    3: [(1, 'A')], 4: [(1, 'B'), ('xc', 1, 0)],
                    5: [(1, 'C'), ('xc', 1, 1)]}
        anchor = None
        for tidx, (b, r0, nrows) in enumerate(SCHEDULE):
            for item in deferred.pop(tidx, []):
                if item[0] == 'xc':
                    load_xc(item[1], item[2], after=anchor)
                else:
                    load_z(item[0], item[1], after=anchor)
            anchor, _ = macro_tile(b, r0, nrows)

        # tail: the last 8 rows run the i+c pair only, epilogue in two
        # 256-col halves interleaved with tiny o-gate slices (each slice's
        # PSUM in a freed pair pool, so the convs never serialize behind a
        # sigmoid). Minimal VectorE work is gated on the final matmuls; the
        # chain after the very last one is sigmoid -> 4 muls -> sub -> DMA.
        last_mm = [None]
        ch, roff = find_chunk(1, 48, 16)
        pk1 = k_group(ps_k1, "pt_k1", 0, 2, 1, 48, 16, roff, ch, last_mm)
        pk3 = k_group(ps_k3, "pt_k3", 9, 1, 1, 48, 16, roff, ch, last_mm)
        pk2 = k_group(ps_k2, "pt_k2", 18, 0, 1, 48, 16, roff, ch, last_mm)
        pks = (pk1, pk2, pk3)
        h0 = [(0, 8) + pair_epilogue(1, 48, pks, 0, 8, sfx="0",
                                     x_on_scalar=True)]
        o_tile(1, 48, 8, 48, h0, pool=ps_k1, tag="pt_k1")
        h1 = [(8, 8) + pair_epilogue(1, 48, pks, 8, 8, sfx="1",
                                     x_on_scalar=True)]
        # NOTE: split_out=True on this last slice (sub+DMA in two halves)
        # was tried and is ~0.9us WORSE: op + trigger overhead beats the
        # smaller final transfer.
        o_tile(1, 56, 8, 48, h1, pool=ps_k2, tag="pt_k2")

    _split_excess_waits(nc)
    return nc


def _prep_inputs(inputs):
    """Host-side shard + layout prep. Returns per-core in_maps."""
    f16 = np.float16
    x = np.asarray(inputs['x'], np.float32)
    h_prev = np.asarray(inputs['h_prev'], np.float32)
    c_prev = np.asarray(inputs['c_prev'], np.float32)

    xr, xi = x[:, :HALF], x[:, HALF:]
    hr, hi = h_prev[:, :HALF], h_prev[:, HALF:]
    cr, ci = c_prev[:, :HALF], c_prev[:, HALF:]

    # combined conv input, channel-major, zero-padded: [128, 3, B, 66, 66]
    # comps: 0 = zr, 1 = zi, 2 = zs = zr + zi (for the Gauss k1 term)
    def prep_z(a, b):
        z = np.concatenate([a, b], axis=1).transpose(1, 0, 2, 3)
        return np.pad(z, ((0, 0), (0, 0), (1, 1), (1, 1)))
    zr = prep_z(xr, hr)
    zi = prep_z(xi, hi)
    zall = np.stack([zr, zi, zr + zi], axis=1).astype(f16)

    # x (*) c_prev (complex elementwise), channel-major fp16: [128, B, 64, 64]
    xc = np.concatenate([xr * cr - xi * ci, xr * ci + xi * cr],
                        axis=1).transpose(1, 0, 2, 3).astype(f16)

    # packed gate weights [cin 128, 45, cout 128] fp16, k-major:
    #  slots 0-8 k1 taps [Wr_i | Wr_c], 9-17 k3 [(Wr+Wi)_i | (Wr+Wi)_c],
    #  18-26 k2 [(Wi-Wr)_i | (Wi-Wr)_c] for the Gauss (i, c) pair;
    #  slots 27 + t*2 + comp for direct o: [Wr|Wi] / [-Wi|Wr]
    W4 = {gn: (np.asarray(inputs['Wr_' + gn], np.float32),
               np.asarray(inputs['Wi_' + gn], np.float32))
          for gn in ('i', 'c', 'o')}
    wts = np.empty((NSLOT, P, P), np.float32)
    for t in range(9):
        kh, kw = t // 3, t % 3
        for gi, gn in enumerate(('i', 'c')):
            Wr, Wi = W4[gn]
            sl = slice(gi * HALF, gi * HALF + HALF)
            wts[t, :, sl] = Wr[:, :, kh, kw].T
            wts[9 + t, :, sl] = (Wr + Wi)[:, :, kh, kw].T
            wts[18 + t, :, sl] = (Wi - Wr)[:, :, kh, kw].T
        Wr, Wi = W4['o']
        wts[O_BASE + t * 2 + 0, :, :HALF] = Wr[:, :, kh, kw].T
        wts[O_BASE + t * 2 + 0, :, HALF:] = Wi[:, :, kh, kw].T
        wts[O_BASE + t * 2 + 1, :, :HALF] = -Wi[:, :, kh, kw].T
        wts[O_BASE + t * 2 + 1, :, HALF:] = Wr[:, :, kh, kw].T
    wts = np.ascontiguousarray(wts.transpose(1, 0, 2)).astype(f16)

    # bias cols: 0 -> sm sources [br_i; br_c], 1 -> sp sources [bi_i; bi_c],
    # 2 -> o gate [br_o; bi_o], 3 -> negated c-Re for CTs [0; -br_c]
    b4 = {k: np.asarray(inputs[k], np.float32)
          for k in ('br_i', 'bi_i', 'br_c', 'bi_c', 'br_o', 'bi_o')}
    bias = np.zeros((P, 4), np.float32)
    bias[:, 0] = np.concatenate([b4['br_i'], b4['br_c']])
    bias[:, 1] = np.concatenate([b4['bi_i'], b4['bi_c']])
    bias[:, 2] = np.concatenate([b4['br_o'], b4['bi_o']])
    bias[HALF:, 3] = -b4['br_c']

    in_maps = []
    for c in range(N_CORES):
        sl = slice(c * B_CORE, (c + 1) * B_CORE)
        in_maps.append({
            "zc": np.ascontiguousarray(zall[:, :, sl]),
            "wts": wts,
            "bias": bias,
            "xc": np.ascontiguousarray(xc[:, sl]),
        })
    return in_maps


def _gather_outputs(results):
    h_full = np.empty((B, P, H, W), np.float32)
    c_full = np.empty((B, P, H, W), np.float32)
    for c in range(N_CORES):
        sl = slice(c * B_CORE, (c + 1) * B_CORE)
        h_full[sl] = results[c]["h_out"].transpose(1, 0, 2, 3).astype(np.float32)
        c_full[sl] = results[c]["c_out"].transpose(1, 0, 2, 3).astype(np.float32)
    return h_full, c_full


def _run(inputs, trace=False, trace_kwargs=None):
    from concourse.bass_utils import run_bass_kernel_spmd

    if "nc" not in _CACHE:
        _CACHE["nc"] = _build_program()
    nc = _CACHE["nc"]
    in_maps = _prep_inputs(inputs)
    r = run_bass_kernel_spmd(nc, in_maps, list(range(N_CORES)),
                             trace=trace, trace_kwargs=trace_kwargs or {})
    return _gather_outputs(r.results), r


def kernel(**inputs):
    (h_full, c_full), _ = _run(inputs)
    return h_full, c_full

